# revision 1
# baseline (speedup 1.0000x reference)
"""Trainium2 Bass kernel for DeformablePatchSampler2d (v5).

out[n, m, c, i, j] = bilinear_sample(x[n, c], row=RY[m, j], col=CX[m, i])

Sampling grid is batch/channel-invariant and known on the host from
`offset`; windows/weights are baked in at build time. Data-parallel over
batch N=8 across 8 cores.

v5 structure (per core):
  - 4 band-PAIRS: partition half s holds band b = p + 4*s (64 channels
    each), so every compute op runs 128 partitions wide.
  - x is pre-cropped AND pre-cast to fp16 on the host into
    x_pack[band, c, 20*span]: per-band row/col windows, contiguous
    ~10.6KB per (band, channel). The DMA fabric runs ~22GB/s/engine at
    that packet size vs ~10 at 500B, and fp16 halves the bytes (the
    rel-err gate is 2e-2; fp16 sampling lands ~5e-4).
  - patch column anchors are spaced exactly 35 px, so slot origins are
    uniform (base + 35*mh, rows rho0=0); per-patch floor jitter is
    absorbed as tap shifts with zero-padded weights. Stage 1 is then
    ONE merged tensor_tensor per row-tap (slot dim stride 35 into the
    band tile) -- DVE op cost is ~flat in elements, so op count rules.
  - stage 2 (col taps) is merged across slots and runs on Pool (its
    bcast-outer mult is 2x faster than DVE's); tap adds stay on DVE.
  - outputs are written [band, c, slot, 16*16] fp16 so each store
    descriptor is 4KB; the host unpermutes, transposes j/i, upcasts.
"""
import numpy as np

_P = 16
_NPH = _NPW = 8
_M = 64
_H = _W = 384
_C = 64
_N = 8
_RW = 20            # rows per band tile
_Q = 20             # stage-1 window width (cols read per slot)
_RT_MAX = 4         # row-tap slots in the weight layout
_CT_MAX = 4         # col-tap slots
_STRIDE = 35        # exact anchor spacing of patch columns
_WSLOT = _RT_MAX * 16 + _CT_MAX * 16   # 128 weight floats per slot
_WPAIR = 8 * _WSLOT                    # 1024 per pair


def _precompute(offset: np.ndarray):
    """Window origins + 3-tap weights, f32 coord math mirroring the reference."""
    offset = offset.astype(np.float32)
    one, half = np.float32(1.0), np.float32(0.5)
    ch = np.linspace(0.0, float(_H), _NPH + 4).astype(np.float32)[2:-2]
    cw = np.linspace(0.0, float(_W), _NPW + 4).astype(np.float32)[2:-2]
    rel = np.arange(_P, dtype=np.float32) - np.float32(_P // 2)
    a = np.arange(_M) // _NPW
    b = np.arange(_M) % _NPW
    hc = ch[a][:, None] + rel[None, :]
    wcen = cw[b][:, None] + rel[None, :]
    gx = (np.float32(2.0) * hc / np.float32(_H - 1) - one) + offset[:, 0:1]
    gy = (np.float32(2.0) * wcen / np.float32(_W - 1) - one) + offset[:, 1:2]
    CX = (((gx + one) * np.float32(_W) - one) * half).astype(np.float64)  # (M,16) cols, dim i
    RY = (((gy + one) * np.float32(_H) - one) * half).astype(np.float64)  # (M,16) rows, dim j

    r0 = np.floor(RY[:, 0]).astype(np.int64)
    c0 = np.floor(CX[:, 0]).astype(np.int64)
    t_r = RY - (r0[:, None] + np.arange(_P)[None, :])
    t_c = CX - (c0[:, None] + np.arange(_P)[None, :])
    assert (t_r >= 0).all() and (t_r < 2).all()
    assert (t_c >= 0).all() and (t_c < 2).all()
    assert r0.min() >= 0 and (r0 + 17).max() <= _H - 1
    assert c0.min() >= 0 and (c0 + 17).max() <= _W - 1

    def taps(t):
        w0 = np.maximum(0.0, 1.0 - t)
        w2 = np.maximum(0.0, t - 1.0)
        return np.stack([w0, 1.0 - w0 - w2, w2], axis=-1).astype(np.float32)

    wr = taps(t_r)  # (M, 16, 3) applies to j (rows)
    wc = taps(t_c)  # (M, 16, 3) applies to i (cols)
    nt_r = np.where(np.abs(wr[:, :, 2]).max(axis=1) > 0, 3, 2)
    nt_c = np.where(np.abs(wc[:, :, 2]).max(axis=1) > 0, 3, 2)
    return r0, c0, wr, wc, nt_r, nt_c


def _plan(offset: np.ndarray):
    r0, c0, wr, wc, nt_r, nt_c = _precompute(offset)
    mw_of = np.arange(_M) % _NPW
    mh_of = np.arange(_M) // _NPW
    band_r0 = np.array([r0[mw_of == b].min() for b in range(8)])
    # uniform slot origins: window of slot mh starts at lo_b + 35*mh
    band_c0 = np.array([(c0 - _STRIDE * mh_of)[mw_of == b].min() for b in range(8)])
    span = int(max(_STRIDE * 7 + _Q,
                   (c0 + 18 - band_c0[mw_of] - 0)[np.arange(_M)].max()))
    span = (span + 1) & ~1
    assert all(r0[m] - band_r0[mw_of[m]] <= 1 for m in range(_M))
    assert band_r0.max() + _RW <= _H
    assert band_c0.min() >= 0 and (band_c0 + span).max() <= _W

    w_all = np.zeros((128, 4 * _WPAIR), dtype=np.float32)
    pairs = []
    for p in range(4):
        bands = (p, p + 4)
        rt_pair, ct_pair = 0, 0
        for mh in range(8):
            for s in range(2):
                m = mh * 8 + bands[s]
                rshift = int(r0[m] - band_r0[bands[s]])
                cshift = int(c0[m] - band_c0[bands[s]] - _STRIDE * mh)
                assert 0 <= rshift <= 1 and 0 <= cshift <= _CT_MAX - 2, \
                    (p, mh, s, rshift, cshift)
                rt_pair = max(rt_pair, rshift + int(nt_r[m]))
                ct_pair = max(ct_pair, cshift + int(nt_c[m]))
                assert cshift + 17 <= _Q and rshift + 17 <= _RW - 1
                base = p * _WPAIR + mh * _WSLOT
                rows = slice(s * 64, (s + 1) * 64)
                wrs = np.zeros((_RT_MAX, 16), dtype=np.float32)
                wcs = np.zeros((_CT_MAX, 16), dtype=np.float32)
                wrs[rshift:rshift + 3] = wr[m].T
                wcs[cshift:cshift + 3] = wc[m].T
                w_all[rows, base:base + _RT_MAX * 16] = wrs.reshape(-1)[None, :]
                w_all[rows, base + _RT_MAX * 16:base + _WSLOT] = \
                    wcs.reshape(-1)[None, :]
        assert rt_pair <= _RT_MAX and ct_pair <= _CT_MAX
        pairs.append(dict(p=p, bands=bands, rt=rt_pair, ct=ct_pair,
                          r0=[int(band_r0[b]) for b in bands],
                          c0=[int(band_c0[b]) for b in bands]))
    return pairs, span, w_all


def _build(pairs, span):
    import concourse.bacc as bacc
    import concourse.mybir as mybir
    from concourse.bass import AP
    from concourse.tile import TileContext

    f16 = mybir.dt.float16
    mult = mybir.AluOpType.mult
    add = mybir.AluOpType.add

    ROWLEN = _RW * span
    TS = 16 * _Q       # t elements per slot (320)

    nc = bacc.Bacc("TRN2", target_bir_lowering=False)
    x_p = nc.dram_tensor("x_pack", (8, _C, ROWLEN), f16, kind="ExternalInput")
    w_d = nc.dram_tensor("w_all", (128, 4 * _WPAIR), f16, kind="ExternalInput")
    out_d = nc.dram_tensor("out_d", (8, _C, 8, 256), f16, kind="ExternalOutput")

    def sub_ap(base_ap, extra_off, free_dims):
        return AP(base_ap.tensor, base_ap.offset + extra_off,
                  [list(base_ap.ap[0])] + [list(d) for d in free_dims])

    with TileContext(nc) as tc:
        with tc.tile_pool(name="fpool", bufs=3) as fpool, \
             tc.tile_pool(name="wpool", bufs=1) as wpool, \
             tc.tile_pool(name="tpool", bufs=2) as tpool, \
             tc.tile_pool(name="mpool", bufs=2) as mpool, \
             tc.tile_pool(name="opool", bufs=2) as opool, \
             tc.tile_pool(name="npool", bufs=2) as npool:
            W_sb = wpool.tile([128, 4 * _WPAIR], f16)
            nc.scalar.dma_start(out=W_sb[:], in_=w_d[:])
            wb = W_sb[:]

            def emit_load(pair):
                F = fpool.tile([128, ROWLEN], f16)
                for s in range(2):
                    src = AP(x_p[:].tensor, pair["bands"][s] * _C * ROWLEN,
                             [[ROWLEN, _C], [1, ROWLEN]])
                    nc.sync.dma_start(out=F[s * 64:(s + 1) * 64, :], in_=src)
                return F

            def emit_store(pair, O):
                for s in range(2):
                    b = pair["bands"][s]
                    dst = AP(out_d[:].tensor, b * (_C * 8 * 256),
                             [[8 * 256, _C], [1, 8 * 256]])
                    nc.scalar.dma_start(out=dst, in_=O[s * 64:(s + 1) * 64, :])

            PREFETCH = 3
            ftiles = {i: emit_load(pairs[i]) for i in range(PREFETCH)}
            pending_store = None
            for pi, pair in enumerate(pairs):
                p = pair["p"]
                bb = ftiles.pop(pi)[:]
                if pi + PREFETCH < len(pairs):
                    ftiles[pi + PREFETCH] = emit_load(pairs[pi + PREFETCH])

                wpair = p * _WPAIR
                T = tpool.tile([128, 8 * TS], f16)
                tb = T[:]
                # stage 1: one merged op per row tap; slot dim strides 35
                # into the band tile, tap k shifts the base row
                for k in range(pair["rt"]):
                    dstt = tb
                    if k > 0:
                        Mt = mpool.tile([128, 8 * TS], f16, name=f"Mt{k}")
                        dstt = Mt[:]
                    dst = sub_ap(dstt, 0, [[TS, 8], [_Q, 16], [1, _Q]])
                    src = sub_ap(bb, k * span, [[_STRIDE, 8], [span, 16], [1, _Q]])
                    w_ap = sub_ap(wb, wpair + k * 16,
                                  [[_WSLOT, 8], [1, 16], [0, _Q]])
                    nc.vector.tensor_tensor(out=dst, in0=src, in1=w_ap, op=mult)
                    if k > 0:
                        nc.vector.tensor_tensor(
                            out=sub_ap(tb, 0, [[1, 8 * TS]]),
                            in0=sub_ap(tb, 0, [[1, 8 * TS]]),
                            in1=sub_ap(Mt[:], 0, [[1, 8 * TS]]),
                            op=add)
                # stage 2: merged col-tap mults on Pool, adds on DVE
                O = opool.tile([128, 8 * 256], f16)
                ob = O[:]
                for ik in range(pair["ct"]):
                    w_ap = sub_ap(wb, wpair + _RT_MAX * 16 + ik * 16,
                                  [[_WSLOT, 8], [0, 16], [1, 16]])
                    in0 = sub_ap(tb, ik, [[TS, 8], [_Q, 16], [1, 16]])
                    if ik == 0:
                        o_ap = sub_ap(ob, 0, [[256, 8], [16, 16], [1, 16]])
                        nc.gpsimd.tensor_tensor(out=o_ap, in0=in0, in1=w_ap,
                                                op=mult)
                    else:
                        MO = npool.tile([128, 8 * 256], f16, name=f"MO{ik}")
                        m_ap = sub_ap(MO[:], 0, [[256, 8], [16, 16], [1, 16]])
                        nc.gpsimd.tensor_tensor(out=m_ap, in0=in0, in1=w_ap,
                                                op=mult)
                        nc.vector.tensor_tensor(
                            out=sub_ap(ob, 0, [[1, 8 * 256]]),
                            in0=sub_ap(ob, 0, [[1, 8 * 256]]),
                            in1=sub_ap(MO[:], 0, [[1, 8 * 256]]),
                            op=add)
                if pending_store is not None:
                    emit_store(*pending_store)
                pending_store = (pair, O)

            emit_store(*pending_store)
    nc.compile()
    return nc


def _prepare(offset):
    pairs, span, w_all = _plan(offset)
    nc = _build(pairs, span)
    aux = dict(pairs=pairs, span=span, w_all=w_all.astype(np.float16))
    return nc, aux


def _pack_x(xn, aux):
    """(C, H, W) f32 -> (8, C, 20*span) fp16 band crops."""
    span = aux["span"]
    out = np.empty((8, _C, _RW * span), dtype=np.float16)
    for p in aux["pairs"]:
        for s in range(2):
            b = p["bands"][s]
            r0, c0 = p["r0"][s], p["c0"][s]
            out[b] = xn[:, r0:r0 + _RW, c0:c0 + span].astype(
                np.float16).reshape(_C, -1)
    return out


def _run(nc, x, aux, **kwargs):
    from concourse.bass_utils import run_bass_kernel_spmd
    in_maps = [{"x_pack": _pack_x(x[n], aux), "w_all": aux["w_all"]}
               for n in range(_N)]
    return run_bass_kernel_spmd(nc, in_maps, core_ids=list(range(_N)), **kwargs)


def _postprocess(out_dev, pairs):
    """out_dev: (8 bands, C, 8 slots, 256) fp16 -> (M, C, 16, 16) f32.

    Device slot layout is [j][i]-major (keeps DVE last dims packed);
    semantic output is [i][j]."""
    out = np.empty((_M, _C, _P, _P), dtype=np.float32)
    for b in range(8):
        for mh in range(8):
            out[mh * 8 + b] = out_dev[b, :, mh].astype(
                np.float32).reshape(_C, _P, _P).transpose(0, 2, 1)
    return out


def kernel(x: np.ndarray, offset: np.ndarray) -> np.ndarray:
    x = np.asarray(x, dtype=np.float32)
    offset = np.asarray(offset, dtype=np.float32)
    nc, aux = _prepare(offset)
    res = _run(nc, x, aux)
    return np.stack([_postprocess(res.results[n]["out_d"], aux["pairs"])
                     for n in range(_N)])



# revision 2
# speedup vs baseline: 1.0084x; 1.0084x over previous
"""Trainium2 Bass kernel for DeformablePatchSampler2d (TensorE matmul formulation).

out[n, m, c, i, j] = bilinear_sample(x[n, c], row=RY[m, j], col=CX[m, i])

The sampling grid is batch/channel-invariant, so per (patch m, i-half a)
the whole bilinear reduces to ONE small dense matmul:

    out[(il, j), (c, n)] = sum_px W[px, (il, j)] * xwin[px, (c, n)]

with px ranging over the patch's 18x10 input window (rows x cols needed
by the 16 j-taps x 8 i-taps) and W = A (row weights, 2-tap) x B (col
weights, 2-tap) built on the host. K = 180 px is split into 2 chunks of
90 that accumulate in PSUM; free dim = (c=64, n=8) = 512 = 1 PSUM bank.

Sharding: SLOTS across cores (core k owns patches 8k..8k+7, all n), so
each matmul streams 512 free elements instead of 64 -- the PE array does
all the arithmetic, DVE/Act only evacuate PSUM->SBUF with an fp16 cast.

Per core: 1 weight DMA + 4 x-window DMAs (720KB each) || 18 HAM-warmup
matmuls || 32 real matmuls || 16 PSUM evacs (alternating Act/DVE) || 4
output DMAs. Everything fp16 on the wire (gate is 2e-2; fp16 ~1e-3).
"""
import numpy as np

_P = 16
_NPH = _NPW = 8
_M = 64
_H = _W = 384
_C = 64
_N = 8
_RWIN = 18          # rows per (m) window (16 j-taps span <=17 rows + 1)
_CWIN = 10          # cols per (m, a) window (8 i-taps span <=9 cols + 1)
_K = 90             # contraction rows per matmul chunk (180 px / 2)
_NITEM = 32         # (slot 8) x (a 2) x (chunk 2) matmuls per core
_NOUT = 16          # (slot 8) x (a 2) psum tiles per core
_FREE = 512         # (c 64) x (n 8)


def _coords(offset: np.ndarray):
    """f32 coordinate math mirroring the reference, then f64 for floors."""
    offset = offset.astype(np.float32)
    one, half = np.float32(1.0), np.float32(0.5)
    ch = np.linspace(0.0, float(_H), _NPH + 4).astype(np.float32)[2:-2]
    cw = np.linspace(0.0, float(_W), _NPW + 4).astype(np.float32)[2:-2]
    rel = np.arange(_P, dtype=np.float32) - np.float32(_P // 2)
    a = np.arange(_M) // _NPW
    b = np.arange(_M) % _NPW
    hc = ch[a][:, None] + rel[None, :]
    wcen = cw[b][:, None] + rel[None, :]
    gx = (np.float32(2.0) * hc / np.float32(_H - 1) - one) + offset[:, 0:1]
    gy = (np.float32(2.0) * wcen / np.float32(_W - 1) - one) + offset[:, 1:2]
    CX = (((gx + one) * np.float32(_W) - one) * half).astype(np.float64)
    RY = (((gy + one) * np.float32(_H) - one) * half).astype(np.float64)
    return CX, RY  # (M, 16) cols (dim i), rows (dim j)


def _plan(offset: np.ndarray):
    CX, RY = _coords(offset)
    ryf = np.floor(RY).astype(np.int64)
    cxf = np.floor(CX).astype(np.int64)
    fr = (RY - ryf).astype(np.float32)
    fc = (CX - cxf).astype(np.float32)
    r0 = ryf[:, 0]                    # (M,) window top row
    c0 = cxf[:, 0::8]                 # (M, 2) window left col per a-half
    rrel = ryf - r0[:, None]
    crel = cxf - np.repeat(c0, 8, axis=1)
    assert rrel.min() >= 0 and (rrel + 1).max() <= _RWIN - 1
    assert crel.min() >= 0 and (crel + 1).max() <= _CWIN - 1
    assert r0.min() >= 0 and (r0 + _RWIN).max() <= _H
    assert c0.min() >= 0 and (c0 + _CWIN).max() <= _W

    A = np.zeros((_M, _RWIN, 16), np.float32)
    mm = np.arange(_M)[:, None]
    jj = np.arange(16)[None, :]
    A[mm, rrel, jj] = 1.0 - fr
    A[mm, rrel + 1, jj] = fr
    B = np.zeros((_M, 2, _CWIN, 8), np.float32)
    il = np.arange(8)[None, :]
    for aa in range(2):
        cr = crel[:, aa * 8:(aa + 1) * 8]
        f = fc[:, aa * 8:(aa + 1) * 8]
        B[mm, aa, cr, il] = 1.0 - f
        B[mm, aa, cr + 1, il] = f
    # W[m, a, (dr, dc), (il, j)]
    W = np.einsum('mrj,maci->marcij', A, B).reshape(_M, 2, _RWIN * _CWIN, 128)
    return dict(r0=r0, c0=c0, W=W.astype(np.float16))


def _build():
    import concourse.bacc as bacc
    import concourse.mybir as mybir
    from concourse.bass import AP
    from concourse.tile import TileContext

    f16 = mybir.dt.float16
    f32 = mybir.dt.float32

    nc = bacc.Bacc("TRN2", target_bir_lowering=False)
    x_p = nc.dram_tensor("x_pack", (_K, _NITEM * _FREE), f16,
                         kind="ExternalInput")
    w_d = nc.dram_tensor("w_pack", (_K, _NITEM * 128), f16,
                         kind="ExternalInput")
    out_d = nc.dram_tensor("out_d", (128, _NOUT * _FREE), f16,
                           kind="ExternalOutput")

    with TileContext(nc) as tc:
        with tc.tile_pool(name="xpool", bufs=4) as xpool, \
             tc.tile_pool(name="wpool", bufs=1) as wpool, \
             tc.tile_pool(name="opool", bufs=4) as opool, \
             tc.tile_pool(name="warm", bufs=1) as warmpool, \
             tc.tile_pool(name="wps", bufs=1, space="PSUM") as wpspool, \
             tc.tile_pool(name="ps", bufs=6, space="PSUM") as pspool:
            W_sb = wpool.tile([_K, _NITEM * 128], f16)
            nc.sync.dma_start(out=W_sb[:], in_=w_d[:])

            # HAM warm-up: keep PE busy during the input DMAs so the real
            # matmuls run at 2.4 GHz instead of the cold 1.2 GHz.
            wu = warmpool.tile([128, _FREE], f16)
            nc.vector.memset(wu[:], 0.0)
            wups = wpspool.tile([128, _FREE], f32)
            for _ in range(18):
                nc.tensor.matmul(wups[:], wu[:, :128], wu[:],
                                 start=True, stop=True)

            xtiles = []
            for g in range(4):
                X = xpool.tile([_K, 8 * _FREE], f16, name="X")
                src = AP(x_p[:].tensor, g * 8 * _FREE,
                         [[_NITEM * _FREE, _K], [1, 8 * _FREE]])
                nc.sync.dma_start(out=X[:], in_=src)
                xtiles.append(X)

            otiles = [opool.tile([128, 4 * _FREE], f16, name="O")
                      for _ in range(4)]
            for it in range(_NOUT):           # it = slot*2 + a
                g, li = divmod(it, 4)         # dma chunk group, local item
                PS = pspool.tile([128, _FREE], f32, name="PS")
                for k in range(2):
                    idx = it * 2 + k
                    nc.tensor.matmul(
                        PS[:],
                        W_sb[:, idx * 128:(idx + 1) * 128],
                        xtiles[g][:, (li * 2 + k) * _FREE:
                                  (li * 2 + k + 1) * _FREE],
                        start=(k == 0), stop=(k == 1))
                dst = otiles[g][:, li * _FREE:(li + 1) * _FREE]
                if it % 2 == 0:
                    nc.scalar.copy(out=dst, in_=PS[:])
                else:
                    nc.vector.tensor_copy(out=dst, in_=PS[:])
                if li == 3:
                    ddst = AP(out_d[:].tensor, g * 4 * _FREE,
                              [[_NOUT * _FREE, 128], [1, 4 * _FREE]])
                    nc.scalar.dma_start(out=ddst, in_=otiles[g][:])
    nc.compile()
    return nc


def _prepare(offset):
    plan = _plan(offset)
    nc = _build()
    return nc, plan


def _pack_core(x16, plan, core):
    """x16: (N, C, H, W) fp16 -> x_pack (90, 32*512), w_pack (90, 32*128)."""
    r0, c0, W = plan["r0"], plan["c0"], plan["W"]
    xp = np.empty((_K, _NITEM, _FREE), np.float16)
    wp = np.empty((_K, _NITEM, 128), np.float16)
    for s in range(8):
        m = core * 8 + s
        for a in range(2):
            win = x16[:, :, r0[m]:r0[m] + _RWIN, c0[m, a]:c0[m, a] + _CWIN]
            win = win.transpose(2, 3, 1, 0).reshape(_RWIN * _CWIN, _FREE)
            for k in range(2):
                it = (s * 2 + a) * 2 + k
                xp[:, it, :] = win[k * _K:(k + 1) * _K]
                wp[:, it, :] = W[m, a, k * _K:(k + 1) * _K]
    return xp.reshape(_K, -1), wp.reshape(_K, -1)


def _run(nc, x, plan, **kwargs):
    from concourse.bass_utils import run_bass_kernel_spmd
    x16 = x.astype(np.float16)
    in_maps = []
    for core in range(_N):
        xp, wp = _pack_core(x16, plan, core)
        in_maps.append({"x_pack": xp, "w_pack": wp})
    return run_bass_kernel_spmd(nc, in_maps, core_ids=list(range(_N)),
                                **kwargs)


def _postprocess(out_dev, core, out):
    """out_dev: (128, 16*512) fp16 for one core -> fills out[:, 8c:8c+8]."""
    r = np.asarray(out_dev).reshape(8, 16, 8, 2, _C, _N)  # il j s a c n
    r = r.transpose(5, 2, 4, 3, 0, 1).astype(np.float32)  # n s c a il j
    out[:, core * 8:(core + 1) * 8] = r.reshape(_N, 8, _C, _P, _P)


def kernel(x: np.ndarray, offset: np.ndarray) -> np.ndarray:
    x = np.asarray(x, dtype=np.float32)
    offset = np.asarray(offset, dtype=np.float32)
    nc, plan = _prepare(offset)
    res = _run(nc, x, plan)
    out = np.empty((_N, _M, _C, _P, _P), np.float32)
    for core in range(_N):
        _postprocess(res.results[core]["out_d"], core, out)
    return out
